# revision 36
# baseline (speedup 1.0000x reference)
"""MiniRocket feature extractor on 8 Trainium2 NeuronCores (blocked fp16).

Per core (4 batch rows), work is organized as 4 dilation-PAIR blocks per
batch row instead of 6 per-dilation units.  A (d, 2d) pair shares tap
positions: the union of both kernels' 9 dilated taps is only 13 offsets
({0,2,..,16} + {5,7,9,11} in d-units), so one [104, L] fp16 tap stack
feeds a matmul whose 128 output partitions hold 84 kernels of one
dilation plus 44 of its partner (504 kernel rows = 4 blocks of <=128).

Per (block, batch) unit:
  - two strided DMAs build the fp16 tap stack [104, 4096] in SBUF
    (even offsets: 72 rows, odd offsets: 32 rows),
  - TensorE: resp = W^T @ xshift (contraction 104, fp16 full rate),
  - ScalarE drains PSUM fp32 -> SBUF int16 with a per-row scale putting
    all four bias thresholds inside +/-32000,
  - VectorE runs 4 fused compare+count passes (is_gt + accum, int16 4x),
  - GpSimd handles the two pad-edge count passes per feature and the
    finalize (feat = full*A - edges*B - C, coefficients host-folded),
  - output DMA on the sync queue (1 per block: feature blocks are
    contiguous in the flat output).
"""

import numpy as np
from contextlib import ExitStack

import concourse.bass as bass
import concourse.mybir as mybir
import concourse.tile as tile
from concourse.ap import AP
from concourse.bass_utils import run_bass_kernel_spmd

DILATIONS = (1, 2, 4, 8, 16, 32)
ND = 6
K = 84
KS = 9
C = 8
L = 4096
F = 4
B = 32
N_CORES = 8
B_LOC = 4
PADMAX = 128
LP = L + 2 * PADMAX
NFEAT = ND * K * F

# (diA, kA0, nA, diB, kB0, nB): block rows = kernels kA0:kA0+nA of
# dilation diA followed by kB0:kB0+nB of diB (= 2*diA's dilation).
BLOCKS = (
    (0, 0, 84, 1, 0, 44),
    (1, 44, 40, 2, 0, 84),
    (3, 0, 84, 4, 0, 44),
    (4, 44, 40, 5, 0, 84),
)
NBLK = 4
XROWS = 104  # 13 tap offsets x 8 channels
CPW = 21     # cpack cols per block: bias 4 | A 4 | BA 4 | BB 4 | C 4 | ss 1

F32 = mybir.dt.float32
F16 = mybir.dt.float16
I16 = mybir.dt.int16


def _split_excess_waits(nc, max_waits=1):
    """This walrus build allows only one sync-wait per instruction; hoist
    extra waits onto preceding NOPs of the same engine."""
    n = 0
    for f in nc.m.functions:
        for bb in f.blocks:
            insts = bb.instructions
            if not any(
                i.sync_info and i.sync_info.on_wait and len(i.sync_info.on_wait) > max_waits
                for i in insts
            ):
                continue
            out = []
            for inst in insts:
                si = inst.sync_info
                waits = list(si.on_wait) if si and si.on_wait else []
                if len(waits) > max_waits:
                    for w in waits[:-max_waits]:
                        nop = mybir.InstNoOp(name=f"syncfix-{n}", ins=[], outs=[])
                        n += 1
                        nop.engine = inst.engine
                        nop.sync_info = mybir.SyncInfo(on_wait=[w], on_update=[])
                        out.append(nop)
                    inst.sync_info = mybir.SyncInfo(
                        on_wait=waits[-max_waits:],
                        on_update=list(si.on_update or []),
                    )
                out.append(inst)
            bb.instructions = out


def _build_nc():
    nc = bass.Bass()
    xprep = nc.declare_dram_parameter("xprep", [B_LOC, C, LP], F16, isOutput=False)
    wstack = nc.declare_dram_parameter("wstack", [NBLK, XROWS, 128], F16, isOutput=False)
    cpack = nc.declare_dram_parameter("cpack", [128, NBLK * CPW], F32, isOutput=False)
    out = nc.declare_dram_parameter("out", [B_LOC, NFEAT], F32, isOutput=True)

    alu = mybir.AluOpType

    with tile.TileContext(nc) as tc, ExitStack() as ctx:
        cpool = ctx.enter_context(tc.tile_pool(name="const", bufs=1))
        xsh_pool = ctx.enter_context(tc.tile_pool(name="xsh", bufs=6))
        psum_pool = ctx.enter_context(tc.tile_pool(name="psum", bufs=2, space="PSUM"))
        resp_pool = ctx.enter_context(tc.tile_pool(name="resp", bufs=5))
        trash_pool = ctx.enter_context(tc.tile_pool(name="trash", bufs=1))
        acc_pool = ctx.enter_context(tc.tile_pool(name="acc", bufs=8))
        feat_pool = ctx.enter_context(tc.tile_pool(name="feat", bufs=8))

        # bass_priority: LOWER value = earlier among ready instructions
        # (default is the program-order counter). Keep each unit's xshift
        # DMA pair adjacent and in unit order, after the small consts.
        def xsh_dmas(xsh, b, blk):
            diA = BLOCKS[blk][0]
            ds = DILATIONS[diA]
            u = b * NBLK + blk
            base = b * C * LP + (PADMAX - 8 * ds)
            if u == 0:
                # column-split the first unit's load so the first matmul
                # half only waits on half the bytes
                for ci, (cc0, cc1) in enumerate(((0, 2048), (2048, L))):
                    i1 = nc.sync.dma_start(
                        xsh[0:72, cc0:cc1],
                        AP(xprep, base + cc0, [[2 * ds, 9], [LP, C], [1, cc1 - cc0]]),
                    )
                    i2 = nc.sync.dma_start(
                        xsh[72:104, cc0:cc1],
                        AP(xprep, base + 5 * ds + cc0, [[2 * ds, 4], [LP, C], [1, cc1 - cc0]]),
                    )
                    i1.ins.bass_priority = -20010 + 2 * ci
                    i2.ins.bass_priority = -20010 + 2 * ci + 1
                return
            i1 = nc.sync.dma_start(
                xsh[0:72, :], AP(xprep, base, [[2 * ds, 9], [LP, C], [1, L]])
            )
            i2 = nc.sync.dma_start(
                xsh[72:104, :], AP(xprep, base + 5 * ds, [[2 * ds, 4], [LP, C], [1, L]])
            )
            i1.ins.bass_priority = -20000 + 2 * u
            i2.ins.bass_priority = -20000 + 2 * u + 1

        # w/cp issue on the Activation queue: keeps the SP sequencer's serial
        # DMA-issue slots for the first xshift load
        w_t = cpool.tile([XROWS, NBLK * 128], F16)
        _iw = nc.scalar.dma_start(
            w_t[:], AP(wstack, 0, [[128, XROWS], [XROWS * 128, NBLK], [1, 128]])
        )
        _iw.ins.bass_priority = -30000

        cp_t = cpool.tile([128, NBLK * CPW], F32, tag="cp_t")
        _ic = nc.scalar.dma_start(cp_t[:], AP(cpack, 0, [[NBLK * CPW, 128], [1, NBLK * CPW]]))
        _ic.ins.bass_priority = -29999

        # Deferred actions (ScalarE sign-count quarters + the dependent
        # finalize) slotted one per later unit, into ScalarE's drain gaps.
        # Deferral must cover every reader of the deferred accums: emitting a
        # reader earlier in trace order than its writer breaks dep tracking.
        pending = []

        trash = trash_pool.tile([128, L], I16)
        trash2 = trash_pool.tile([128, L], I16, tag="trash2")
        trash3 = trash_pool.tile([128, L], I16, tag="trash3")
        # separate scratch for the second edge group: a shared tile would put
        # a WAW sem-wait between every edgeA/edgeB pair
        trash4 = trash_pool.tile([128, 512], I16, tag="trash4")

        # PE warmup: dummy matmuls ramp the tensor engine's p-state while the
        # first xshift DMA is in flight, so the first real matmuls run fast.
        dummy = cpool.tile([XROWS, 512], F16, tag="dummy")
        nc.gpsimd.memset(dummy[:], 0.0)
        ps_warm = psum_pool.tile([128, 2048], F32, tag="ps")
        for n in range(8):
            nc.tensor.matmul(
                ps_warm[:, (n % 4) * 512 : (n % 4 + 1) * 512],
                dummy[:, 0:128], dummy[:], start=True, stop=True,
            )
        # read the warm tile so the pool releases its PSUM buffer
        nc.scalar.activation(
            trash3[:, 0:512], ps_warm[:, 0:512],
            mybir.ActivationFunctionType.Copy,
        )

        for b in range(B_LOC):
            for blk in range(NBLK):
                diA, kA0, nA, diB, kB0, nB = BLOCKS[blk]
                ds = DILATIONS[diA]
                nrows = nA + nB
                c0 = blk * CPW
                bias_t = cp_t[:, c0 : c0 + 4]
                a_t = cp_t[:, c0 + 4 : c0 + 8]
                ba_t = cp_t[:, c0 + 8 : c0 + 12]
                bb_t = cp_t[:, c0 + 12 : c0 + 16]
                cc_t = cp_t[:, c0 + 16 : c0 + 20]
                ss_t = cp_t[:, c0 + 20 : c0 + 21]
                w_blk = w_t[:, blk * 128 : (blk + 1) * 128]

                u = b * NBLK + blk
                xsh = xsh_pool.tile([XROWS, L], F16)
                xsh_dmas(xsh, b, blk)

                resp16 = resp_pool.tile([128, L], I16)
                for h in range(2):
                    ps = psum_pool.tile([128, 2048], F32, tag="ps")
                    for n in range(4):
                        nc.tensor.matmul(
                            ps[:, n * 512 : (n + 1) * 512],
                            w_blk,
                            xsh[:, h * 2048 + n * 512 : h * 2048 + (n + 1) * 512],
                            start=True,
                            stop=True,
                        )
                    nc.scalar.activation(
                        resp16[:, h * 2048 : (h + 1) * 2048], ps[:],
                        mybir.ActivationFunctionType.Copy,
                        scale=ss_t,
                    )
                if pending:
                    pending.pop(0)()

                acc = acc_pool.tile([128, 4 * F + 3], F32)
                pstep = resp16[:].ap[0][0]
                tstep = trash2[:].ap[0][0]
                # engine partition ranges must be 32-aligned, so both edge
                # passes (pad_A and pad_B) run over all 128 rows into separate
                # acc columns; host-built coefs pick the right one per row
                # (BA zero on B rows, BB zero on A rows).
                halves = 2 if u in (0, B_LOC * NBLK - 1) else 1
                for f in range(F):
                    b_ap = bias_t[:, f : f + 1]
                    if blk == 2 and f == 0:
                        # full count on ScalarE: sum of sign(bias - resp) in
                        # four deferred quarter-passes (accum cols 0,16,17,18);
                        # finalize coefs are host-adjusted (A -> -A/2 etc.)
                        def sign_q(resp_t, bb, acc_ap, cc0, cc1):
                            def run():
                                nc.scalar.activation(
                                    trash3[:, cc0:cc1], resp_t[:, cc0:cc1],
                                    mybir.ActivationFunctionType.Sign,
                                    bias=bb, scale=-1.0,
                                    accum_out=acc_ap,
                                )
                            return run

                        for q in range(4):
                            col = f if q == 0 else 4 * F + q - 1
                            pending.append(
                                sign_q(resp16, b_ap, acc[:, col : col + 1],
                                       q * 1024, (q + 1) * 1024)
                            )
                    elif halves == 2:
                        # first/last unit: compare per drained half so the
                        # pipeline starts (finishes) earlier; half-counts land
                        # in cols f and 3F+f and are summed before finalize
                        for hh in range(2):
                            col = f if hh == 0 else 3 * F + f
                            nc.vector.tensor_scalar(
                                trash[:, hh * 2048 : (hh + 1) * 2048],
                                resp16[:, hh * 2048 : (hh + 1) * 2048],
                                b_ap, None, alu.is_gt, alu.add,
                                accum_out=acc[:, col : col + 1],
                            )
                    else:
                        nc.vector.tensor_scalar(
                            trash[:], resp16[:], b_ap, None, alu.is_gt, alu.add,
                            accum_out=acc[:, f : f + 1],
                        )
                    for gi, pad in enumerate((4 * ds, 8 * ds)):
                        tdst = trash2 if gi == 0 else trash4
                        tdstep = tdst[:].ap[0][0]
                        ein = AP(
                            resp16[:].tensor, resp16[:].offset,
                            [[pstep, 128], [L - pad, 2], [1, pad]],
                        )
                        eout = AP(
                            tdst[:].tensor, tdst[:].offset,
                            [[tdstep, 128], [pad, 2], [1, pad]],
                        )
                        col = (1 + gi) * F + f
                        nc.vector.tensor_scalar(
                            eout, ein, b_ap, None,
                            alu.is_gt, alu.add,
                            accum_out=acc[:, col : col + 1],
                        )

                # finalize: feat = full*A - eA*BA - eB*BB - C. Normally on
                # GpSimd; the last unit finalizes on VectorE (no cross-engine
                # hop after its last edge pass) with the out DMA on the idle
                # sync queue. blk2's finalize is deferred behind its sign
                # quarters (its accum writers come later in trace order).
                def finalize(acc, a_t, ba_t, bb_t, cc_t, b, blk, u, halves, nrows):
                    def run():
                        eng = nc.vector if u == B_LOC * NBLK - 1 else nc.gpsimd
                        if halves == 2:
                            eng.tensor_tensor(
                                acc[:, 0:F], acc[:, 0:F], acc[:, 3 * F : 4 * F],
                                alu.add,
                            )
                        if blk == 2:
                            # combine the four sign-quarter accums into col 0
                            for q in range(3):
                                eng.tensor_tensor(
                                    acc[:, 0:1], acc[:, 0:1],
                                    acc[:, 4 * F + q : 4 * F + q + 1], alu.add,
                                )
                        uu = feat_pool.tile([128, F], F32)
                        eng.tensor_mul(uu[:], acc[:, 0:F], a_t)
                        w2 = feat_pool.tile([128, F], F32)
                        eng.tensor_mul(w2[:], acc[:, F : 2 * F], ba_t)
                        w3 = feat_pool.tile([128, F], F32)
                        eng.tensor_mul(w3[:], acc[:, 2 * F : 3 * F], bb_t)
                        ft = feat_pool.tile([128, F], F32)
                        eng.tensor_sub(ft[:], uu[:], w2[:])
                        ft2 = feat_pool.tile([128, F], F32)
                        eng.tensor_sub(ft2[:], ft[:], w3[:])
                        fn = feat_pool.tile([128, F], F32)
                        eng.tensor_sub(fn[:], ft2[:], cc_t)

                        featbase = (0, 512, 1008, 1520)[blk]
                        dst = AP(out, b * NFEAT + featbase, [[F, nrows], [1, F]])
                        if u == B_LOC * NBLK - 1:
                            nc.sync.dma_start(dst, fn[0:nrows, :])
                        else:
                            nc.gpsimd.dma_start(dst, fn[0:nrows, :])
                    return run

                fin = finalize(acc, a_t, ba_t, bb_t, cc_t, b, blk, u, halves, nrows)
                if blk == 2:
                    pending.append(fin)
                else:
                    fin()

        while pending:
            pending.pop(0)()

    _split_excess_waits(nc)
    return nc


_NC_CACHE = None


def _get_nc():
    global _NC_CACHE
    if _NC_CACHE is None:
        _NC_CACHE = _build_nc()
    return _NC_CACHE


LAST_RESULTS = None


def kernel(x, channel_masks, bias_matrices, feature_mean, feature_std):
    global LAST_RESULTS
    x = np.ascontiguousarray(np.asarray(x, dtype=np.float32))
    masks = np.asarray(channel_masks, dtype=np.float32)
    biasm = np.asarray(bias_matrices, dtype=np.float32)
    mean = np.asarray(feature_mean, dtype=np.float32).reshape(ND, K, F)
    std = np.asarray(feature_std, dtype=np.float32).reshape(ND, K, F)

    # int16 drain: resp stored as round(resp * s_dk); thresholds scaled to
    # sit inside +/-32000 (saturation is count-safe beyond max|bias|).
    maxb = np.maximum(np.abs(biasm).max(axis=-1), 1e-6)  # [ND, K]
    sscale = 32000.0 / maxb

    wstack = np.zeros((NBLK, XROWS, 128), np.float16)
    cpk = np.zeros((128, NBLK * CPW), np.float32)
    # even tap offsets u=2i (i=0..8) -> rows i*8+c; odd u=5+2i (i=0..3)
    # -> rows 72+i*8+c  (u in ds units, absolute shift u*ds - 8*ds)
    for blk, (diA, kA0, nA, diB, kB0, nB) in enumerate(BLOCKS):
        rows_for = {}
        for uoff in range(0, 17, 2):
            rows_for[uoff] = (uoff // 2) * 8
        for i, uoff in enumerate((5, 7, 9, 11)):
            rows_for[uoff] = 72 + i * 8
        for j in range(128):
            if j < nA:
                di, k = diA, kA0 + j
                taps = range(4, 13)         # (jj+4) for jj=0..8
            elif j < nA + nB:
                di, k = diB, kB0 + (j - nA)
                taps = range(0, 17, 2)      # 2*jj
            else:
                cpk[j, blk * CPW : blk * CPW + 4] = 32100.0
                cpk[j, blk * CPW + 20] = 1.0
                continue
            for uoff in taps:
                r = rows_for[uoff]
                wstack[blk, r : r + 8, j] = -masks[di, k]
            s = sscale[di, k]
            pad = 4 * DILATIONS[di]
            lt = L - 2 * pad
            par = (di + k) % 2 == 1
            ca = (1.0 / lt if par else 1.0 / L) / std[di, k]
            cb = (1.0 / lt if par else 0.0) / std[di, k]
            cpk[j, blk * CPW : blk * CPW + 4] = biasm[di, k] * s
            cpk[j, blk * CPW + 4 : blk * CPW + 8] = ca
            # edge coef lands in the BA slot for A rows, BB slot for B rows
            boff = 8 if j < nA else 12
            cpk[j, blk * CPW + boff : blk * CPW + boff + 4] = cb
            cpk[j, blk * CPW + 16 : blk * CPW + 20] = mean[di, k] / std[di, k]
            cpk[j, blk * CPW + 20] = s

    # (blk2, f0) full counts come from ScalarE as S = sum(sign(bias - resp)):
    # count = (L - S)/2, so feat = S*(-A/2) - edges*B - (C - L*A/2)
    c0 = 2 * CPW
    a0 = cpk[:, c0 + 4].copy()
    cpk[:, c0 + 16] -= L * a0 / 2.0
    cpk[:, c0 + 4] = -a0 / 2.0

    xt = np.ascontiguousarray(x.transpose(0, 2, 1))
    xp = np.zeros((B, C, LP), np.float16)
    xp[:, :, PADMAX : PADMAX + L] = xt.astype(np.float16)

    nc = _get_nc()
    in_maps = []
    for core in range(N_CORES):
        in_maps.append(
            {
                "xprep": np.ascontiguousarray(xp[core * B_LOC : (core + 1) * B_LOC]),
                "wstack": wstack,
                "cpack": cpk,
            }
        )
    res = run_bass_kernel_spmd(nc, in_maps, list(range(N_CORES)))
    LAST_RESULTS = res
    out = np.concatenate([res.results[i]["out"] for i in range(N_CORES)], axis=0)
    return out.astype(np.float32)
